# revision 1
# baseline (speedup 1.0000x reference)
"""MoE layer (SwiGLU experts, top-2 routing) on 8 Trainium2 NeuronCores.

Strategy (per the expert-parallel sharding hint):
  Launch A (data-parallel): each core takes 1/8 of the tokens and computes
    the router on-device: logits = x @ router_w.T, top-2, softmax ->
    dense combine-weight matrix slice [512, E] (zeros for unrouted experts).
  Host: builds the dispatch (the "all-to-all"): per-expert token index
    lists from the device-computed combine weights, padded to a common
    capacity.
  Launch B (expert-parallel): core e owns expert e's weights. It gathers
    its routed tokens from x via indirect DMA, computes
    silu(x@wg) * (x@wu) @ wd in fp32r on the tensor engine, scales rows by
    the combine weight, and writes the transposed result [D, cap].
  Host: scatter-adds the per-expert results back into the [B, S, D] output.

kernel() is self-contained: shapes/sharding are hardcoded for
  x[2, 2048, 512], router_w[8, 512], w_gate[8, 512, 1024],
  w_up[8, 512, 1024], w_down[8, 1024, 512].
"""
import numpy as np

import concourse.bass as bass
import concourse.mybir as mybir
import concourse.tile as tile
from concourse import bacc
from concourse.bass_utils import run_bass_kernel_spmd
from concourse.masks import make_identity

P = 128
B, S, D, H, E, TOPK = 2, 2048, 512, 1024, 8, 2
S_TOT = B * S            # 4096 tokens
N_CORES = 8
SHARD = S_TOT // N_CORES  # 512 tokens per core in the router launch
KD = D // P               # 4 k-tiles over D
KH = H // P               # 8 k-tiles over H

F32 = mybir.dt.float32
F32R = mybir.dt.float32r
I32 = mybir.dt.int32
AF = mybir.ActivationFunctionType

_router_nc = None
_expert_nc = {}


def _chunks_of(cap):
    """Split cap into chunks: multiples of 128, as even as possible, <=512."""
    n = (cap + 511) // 512
    base = cap // n
    base -= base % P
    sizes = [base] * n
    extra = cap - base * n
    i = 0
    while extra > 0:
        sizes[i] += P
        extra -= P
        i = (i + 1) % n
    out, n0 = [], 0
    for sz in sizes:
        out.append((n0, sz))
        n0 += sz
    return out


def _build_router():
    """Per core: xt [D, SHARD] f32 (transposed token shard), rwt [D, E] f32
    -> dw [SHARD, E] f32 combine weights (0 for unrouted experts)."""
    nc = bacc.Bacc(None, target_bir_lowering=False)
    xt = nc.dram_tensor("xt", [D, SHARD], F32, kind="ExternalInput")
    rwt = nc.dram_tensor("rwt", [D, E], F32, kind="ExternalInput")
    dw = nc.dram_tensor("dw", [SHARD, E], F32, kind="ExternalOutput")

    M4 = SHARD // P  # 4 token groups of 128
    with tile.TileContext(nc) as tc:
        with tc.tile_pool(name="sb", bufs=1) as sb, \
             tc.tile_pool(name="wk", bufs=1) as wk, \
             tc.tile_pool(name="ps", bufs=2, space="PSUM") as ps:
            ident = sb.tile([P, P], F32)
            make_identity(nc, ident[:])
            # PE warmup while DMAs land: keeps HAM at full clock for the
            # real matmuls.
            warm = sb.tile([P, 512], F32)
            nc.gpsimd.memset(warm[:], 0.0)
            for w in range(2):
                psw = ps.tile([P, 384], F32, tag="psw", name=f"psw{w}")
                nc.tensor.matmul(psw[:], warm[:, :P], warm[:, :384], start=True, stop=True)

            rwt_t = sb.tile([P, KD, E], F32)
            nc.sync.dma_start(out=rwt_t[:], in_=rwt.rearrange("(k p) e -> p k e", p=P))
            xt_t = sb.tile([P, KD, SHARD], F32)
            xt_r = xt.rearrange("(k p) n -> p k n", p=P)
            for k in range(KD):
                nc.sync.dma_start(out=xt_t[:, k, :], in_=xt_r[:, k, :])

            # logitsT [E, SHARD] with tiny stationary operand (rwt)
            psl = ps.tile([E, SHARD], F32, tag="psl")
            for k in range(KD):
                nc.tensor.matmul(psl[:], rwt_t[:, k, :], xt_t[:, k, :],
                                 start=(k == 0), stop=(k == KD - 1))
            lgT = sb.tile([E, SHARD], F32)
            nc.scalar.activation(lgT[:], psl[:], AF.Copy)

            # transpose to [tok, E] groups: lg_all[p, m, e] = logits[m*128+p, e]
            lg_all = sb.tile([P, M4, E], F32)
            for m in range(M4):
                pst = ps.tile([P, E], F32, tag="pst")
                nc.tensor.transpose(pst[:], lgT[:, m * P:(m + 1) * P], ident[:E, :E])
                nc.vector.tensor_copy(lg_all[:, m, :], pst[:])

            # batched top-2 softmax combine weights over all 512 tokens
            m1 = wk.tile([P, M4, 1], F32)
            nc.vector.tensor_reduce(m1[:], lg_all[:], axis=mybir.AxisListType.X,
                                    op=mybir.AluOpType.max)
            msk1 = wk.tile([P, M4, E], F32)
            nc.vector.tensor_tensor(out=msk1[:], in0=lg_all[:],
                                    in1=m1[:].to_broadcast([P, M4, E]),
                                    op=mybir.AluOpType.is_equal)
            lg2 = wk.tile([P, M4, E], F32)
            nc.vector.scalar_tensor_tensor(
                out=lg2[:], in0=msk1[:], scalar=-1e30, in1=lg_all[:],
                op0=mybir.AluOpType.mult, op1=mybir.AluOpType.add)
            m2 = wk.tile([P, M4, 1], F32)
            nc.vector.tensor_reduce(m2[:], lg2[:], axis=mybir.AxisListType.X,
                                    op=mybir.AluOpType.max)
            lgm = wk.tile([P, M4, E], F32)
            nc.vector.tensor_sub(lgm[:], lg_all[:], m1[:].to_broadcast([P, M4, E]))
            et = wk.tile([P, M4, E], F32)
            nc.scalar.activation(et[:], lgm[:], AF.Exp)
            edm = wk.tile([P, M4, 1], F32)
            nc.vector.tensor_sub(edm[:], m2[:], m1[:])
            ed = wk.tile([P, M4, 1], F32)
            nc.scalar.activation(ed[:], edm[:], AF.Exp)
            nc.vector.tensor_scalar_add(ed[:], ed[:], 1.0)
            rcp = wk.tile([P, M4, 1], F32)
            nc.vector.reciprocal(rcp[:], ed[:])
            msk = wk.tile([P, M4, E], F32)
            nc.vector.tensor_tensor(out=msk[:], in0=lg_all[:],
                                    in1=m2[:].to_broadcast([P, M4, E]),
                                    op=mybir.AluOpType.is_ge)
            wout = wk.tile([P, M4, E], F32)
            nc.vector.tensor_mul(wout[:], et[:], msk[:])
            nc.vector.tensor_mul(wout[:], wout[:], rcp[:].to_broadcast([P, M4, E]))
            nc.sync.dma_start(out=dw.rearrange("(m p) e -> p m e", p=P), in_=wout[:])
    nc.compile()
    return nc


def _build_expert(cap):
    """Per core (expert e): gather `cap` routed token rows of x, run the
    SwiGLU expert in fp32r, scale by combine weight, emit yt [D, cap]."""
    nc = bacc.Bacc(None, target_bir_lowering=False)
    x = nc.dram_tensor("x", [S_TOT, D], F32, kind="ExternalInput")
    idx = nc.dram_tensor("idx", [cap], I32, kind="ExternalInput")
    wtb = nc.dram_tensor("wtb", [P, cap], F32, kind="ExternalInput")
    wg = nc.dram_tensor("wg", [D, H], F32R, kind="ExternalInput")
    wu = nc.dram_tensor("wu", [D, H], F32R, kind="ExternalInput")
    wd = nc.dram_tensor("wd", [H, D], F32R, kind="ExternalInput")
    yt = nc.dram_tensor("yt", [D, cap], F32, kind="ExternalOutput")

    nt = cap // P
    chunks = _chunks_of(cap)

    with tile.TileContext(nc) as tc:
        with tc.tile_pool(name="const", bufs=1) as const, \
             tc.tile_pool(name="wts", bufs=1) as wts, \
             tc.tile_pool(name="xp", bufs=3) as xp, \
             tc.tile_pool(name="gp", bufs=1) as gp, \
             tc.tile_pool(name="ap", bufs=2) as ap, \
             tc.tile_pool(name="ps_gu", bufs=2, space="PSUM") as ps_gu, \
             tc.tile_pool(name="ps_y", bufs=1, space="PSUM") as ps_y:

            ident = const.tile([P, P], F32)
            make_identity(nc, ident[:])
            idx_t = const.tile([P, nt], I32)
            nc.sync.dma_start(out=idx_t[:], in_=idx.rearrange("(t p) -> p t", p=P))

            # PE warmup while weights/gathers land: dummy matmuls keep HAM at
            # full clock until the first transposes are ready (~16us in).
            warm = const.tile([P, 512], F32)
            nc.gpsimd.memset(warm[:], 0.0)
            for w in range(22):
                psw = ps_gu.tile([P, 256], F32, tag="psg", name=f"psw{w}")
                nc.tensor.matmul(psw[:], warm[:, :P], warm[:, :256], start=True, stop=True)

            # all gathers issued upfront on the SWDGE queue: it streams token
            # rows in parallel with the weight DMAs on HWDGE. (for
            # pathologically unbalanced routing, fall back to a rolling window)
            gts = {}
            upfront = nt if nt <= 16 else 0
            for t in range(upfront):
                g = gp.tile([P, D], F32, tag=f"g{t}", name=f"g{t}")
                nc.gpsimd.indirect_dma_start(
                    out=g[:], out_offset=None, in_=x[:, :],
                    in_offset=bass.IndirectOffsetOnAxis(ap=idx_t[:, t:t + 1], axis=0))
                gts[t] = g

            # weights, loaded in slices ordered by first use
            wg_t = wts.tile([P, KD, H], F32R)
            wu_t = wts.tile([P, KD, H], F32R)
            wd_t = wts.tile([P, KH, D], F32R)
            wg_r = wg.rearrange("(k p) h -> p k h", p=P)
            wu_r = wu.rearrange("(k p) h -> p k h", p=P)
            wd_r = wd.rearrange("(k p) d -> p k d", p=P)
            # weight slices arrive in the order the chunk-0 h-loop consumes
            # them: per j, h-tiles {2j, 2j+1} of wg, wu and wd.
            HSL = 256  # H columns (2 h-tiles) per wg/wu DMA slice
            wtb_t = const.tile([P, cap], F32)
            for j in range(H // HSL):
                hs = slice(j * HSL, (j + 1) * HSL)
                nc.sync.dma_start(out=wg_t[:, :, hs], in_=wg_r[:, :, hs])
                nc.sync.dma_start(out=wu_t[:, :, hs], in_=wu_r[:, :, hs])
                nc.sync.dma_start(out=wd_t[:, 2 * j:2 * j + 2, :],
                                  in_=wd_r[:, 2 * j:2 * j + 2, :])
                if j == 0:
                    nc.sync.dma_start(out=wtb_t[:], in_=wtb[:, :])

            for (n0, nsz) in chunks:
                # stage 1: transpose gathered rows to [D, tok] fp32r
                xT = xp.tile([P, KD, nsz], F32R, tag="xT")
                for tl in range(nsz // P):
                    t = n0 // P + tl
                    if t not in gts:
                        g = gp.tile([P, D], F32, tag=f"g{t % 4}", name=f"g{t}")
                        nc.gpsimd.indirect_dma_start(
                            out=g[:], out_offset=None, in_=x[:, :],
                            in_offset=bass.IndirectOffsetOnAxis(
                                ap=idx_t[:, t:t + 1], axis=0))
                        gts[t] = g
                    g = gts[t]
                    for k in range(KD):
                        pst = ps_gu.tile([P, P], F32, tag="psg", name=f"pst_{t}_{k}")
                        nc.tensor.transpose(pst[:], g[:, k * P:(k + 1) * P], ident[:])
                        # copy-cast on ScalarE: DVE is busy with act muls
                        nc.scalar.activation(xT[:, k, tl * P:(tl + 1) * P], pst[:],
                                             AF.Copy)
                    if t < 6:
                        # filler keeps the PE clock up while gathers pace the
                        # transpose phase
                        psf = ps_gu.tile([P, P], F32, tag="psu", name=f"psf_{t}")
                        nc.tensor.matmul(psf[:], warm[:, :P], warm[:, :P],
                                         start=True, stop=True)

                # stage 2: SwiGLU over this chunk of tokens
                psy = [ps_y.tile([P, nsz], F32, tag=f"psy{d}", name=f"psy{d}_{n0}")
                       for d in range(KD)]
                for h in range(KH):
                    psg = ps_gu.tile([P, nsz], F32, tag="psg")
                    psu = ps_gu.tile([P, nsz], F32, tag="psu")
                    for k in range(KD):
                        nc.tensor.matmul(
                            psg[:], wg_t[:, k, h * P:(h + 1) * P], xT[:, k, :],
                            start=(k == 0), stop=(k == KD - 1))
                    for k in range(KD):
                        nc.tensor.matmul(
                            psu[:], wu_t[:, k, h * P:(h + 1) * P], xT[:, k, :],
                            start=(k == 0), stop=(k == KD - 1))
                    actg = ap.tile([P, nsz], F32, tag="actg")
                    nc.scalar.activation(actg[:], psg[:], AF.Silu)
                    act = ap.tile([P, nsz], F32R, tag="act")
                    nc.vector.tensor_mul(act[:], actg[:], psu[:])
                    for d in range(KD):
                        nc.tensor.matmul(
                            psy[d][:], wd_t[:, h, d * P:(d + 1) * P], act[:],
                            start=(h == 0), stop=(h == KH - 1))
                for d in range(KD):
                    yts = ap.tile([P, nsz], F32, tag="yts")
                    nc.vector.tensor_mul(yts[:], psy[d][:], wtb_t[:, n0:n0 + nsz])
                    nc.sync.dma_start(out=yt[d * P:(d + 1) * P, n0:n0 + nsz],
                                      in_=yts[:])
    nc.compile()
    return nc


def _get_router_nc():
    global _router_nc
    if _router_nc is None:
        _router_nc = _build_router()
    return _router_nc


def _get_expert_nc(cap):
    if cap not in _expert_nc:
        _expert_nc[cap] = _build_expert(cap)
    return _expert_nc[cap]


def kernel(x, router_w, w_gate, w_up, w_down, _timings=None):
    x = np.ascontiguousarray(x, dtype=np.float32)
    router_w = np.ascontiguousarray(router_w, dtype=np.float32)
    w_gate = np.ascontiguousarray(w_gate, dtype=np.float32)
    w_up = np.ascontiguousarray(w_up, dtype=np.float32)
    w_down = np.ascontiguousarray(w_down, dtype=np.float32)

    flat = x.reshape(S_TOT, D)
    rwt = np.ascontiguousarray(router_w.T)  # [D, E]

    # ---- Launch A: router (data-parallel over token shards) ----
    nc_a = _get_router_nc()
    in_maps_a = []
    for c in range(N_CORES):
        sh = flat[c * SHARD:(c + 1) * SHARD]  # [SHARD, D]
        in_maps_a.append({"xt": np.ascontiguousarray(sh.T), "rwt": rwt})
    res_a = run_bass_kernel_spmd(nc_a, in_maps_a, core_ids=list(range(N_CORES)))
    dw = np.concatenate([res_a.results[c]["dw"] for c in range(N_CORES)], axis=0)
    if _timings is not None:
        _timings["router_ns"] = res_a.exec_time_ns

    # ---- Host: build the dispatch (all-to-all by expert) ----
    sel = dw > 0.0
    idx_list = [np.nonzero(sel[:, e])[0].astype(np.int32) for e in range(E)]
    counts = [len(ix) for ix in idx_list]
    cap = max(max(counts), 1)
    cap = ((cap + P - 1) // P) * P

    in_maps_b = []
    for e in range(E):
        ix = idx_list[e]
        wt = np.zeros(cap, dtype=np.float32)
        wt[:len(ix)] = dw[ix, e]
        wtb = np.ascontiguousarray(np.broadcast_to(wt[None, :], (P, cap)))
        pad = np.zeros(cap, dtype=np.int32)
        pad[:len(ix)] = ix
        in_maps_b.append({
            "x": flat,
            "idx": pad,
            "wtb": wtb,
            "wg": w_gate[e],
            "wu": w_up[e],
            "wd": w_down[e],
        })

    # ---- Launch B: experts (expert-parallel) ----
    nc_b = _get_expert_nc(cap)
    res_b = run_bass_kernel_spmd(nc_b, in_maps_b, core_ids=list(range(N_CORES)))
    if _timings is not None:
        _timings["expert_ns"] = res_b.exec_time_ns

    # ---- Host: combine (scatter-add back, then unshard) ----
    out = np.zeros((S_TOT, D), dtype=np.float32)
    for e in range(E):
        ix = idx_list[e]
        if len(ix) == 0:
            continue
        ytc = res_b.results[e]["yt"][:, :len(ix)]  # [D, cnt]
        out[ix] += ytc.T  # indices unique within an expert
    return out.reshape(B, S, D)



# revision 2
# speedup vs baseline: 1.2229x; 1.2229x over previous
"""MoE layer (SwiGLU experts, top-2 routing) on 8 Trainium2 NeuronCores.

Strategy (per the expert-parallel sharding hint):
  Launch A (data-parallel): each core takes 1/8 of the tokens and computes
    the router on-device in fp32 (top-2 selection needs fp32: the min
    2nd/3rd logit gap is ~2.6e-4): logits = x @ router_w.T token-major,
    top-2, softmax -> dense combine-weight matrix slice [512, E].
  Host: builds the dispatch (the "all-to-all"): per-expert token lists,
    gathers + transposes + bf16-casts the routed token rows, pads to a
    common capacity.
  Launch B (expert-parallel): core e owns expert e's weights (bf16). It
    computes silu(x@wg) * (x@wu) @ wd in bf16 on the tensor engine (bf16
    streams the moving operand at 1 elem/lane/cycle vs 1/2 for fp32r, so
    matmuls run 2x faster warm), scales rows by the combine weight, and
    writes the result [D, cap] in fp32.
  Host: scatter-adds the per-expert results back into the [B, S, D] output.

kernel() is self-contained: shapes/sharding are hardcoded for
  x[2, 2048, 512], router_w[8, 512], w_gate[8, 512, 1024],
  w_up[8, 512, 1024], w_down[8, 1024, 512].
"""
import numpy as np
import ml_dtypes

import concourse.bass as bass
import concourse.mybir as mybir
import concourse.tile as tile
from concourse import bacc
from concourse.bass_utils import run_bass_kernel_spmd

P = 128
B, S, D, H, E, TOPK = 2, 2048, 512, 1024, 8, 2
S_TOT = B * S            # 4096 tokens
N_CORES = 8
SHARD = S_TOT // N_CORES  # 512 tokens per core in the router launch
KD = D // P               # 4 k-tiles over D
KH = H // P               # 8 k-tiles over H

F32 = mybir.dt.float32
BF16 = mybir.dt.bfloat16
NP_BF16 = ml_dtypes.bfloat16
AF = mybir.ActivationFunctionType

_router_nc = None
_expert_nc = {}


def _chunks_of(cap):
    """Split cap into chunks: multiples of 128, as even as possible, <=512."""
    n = (cap + 511) // 512
    base = cap // n
    base -= base % P
    sizes = [base] * n
    extra = cap - base * n
    i = 0
    while extra > 0:
        sizes[i] += P
        extra -= P
        i = (i + 1) % n
    out, n0 = [], 0
    for sz in sizes:
        out.append((n0, sz))
        n0 += sz
    return out


def _build_router():
    """Per core: xt [D, SHARD] f32 (transposed token shard), rwt [D, E] f32
    -> dw [SHARD, E] f32 combine weights (0 for unrouted experts).

    Logits are computed token-major (stationary = x k-tile, moving = rwt)
    so no transposes are needed before the top-2/softmax vector chain.
    """
    nc = bacc.Bacc(None, target_bir_lowering=False)
    xt = nc.dram_tensor("xt", [D, SHARD], F32, kind="ExternalInput")
    rwt = nc.dram_tensor("rwt", [D, E], F32, kind="ExternalInput")
    dw = nc.dram_tensor("dw", [SHARD, E], F32, kind="ExternalOutput")

    M4 = SHARD // P  # 4 token groups of 128
    with tile.TileContext(nc) as tc:
        with tc.tile_pool(name="sb", bufs=1) as sb, \
             tc.tile_pool(name="wk", bufs=1) as wk, \
             tc.tile_pool(name="ps", bufs=1, space="PSUM") as ps:
            rwt_t = sb.tile([P, KD, E], F32)
            nc.sync.dma_start(out=rwt_t[:], in_=rwt.rearrange("(k p) e -> p k e", p=P))
            xt_t = sb.tile([P, KD, SHARD], F32)
            xt_r = xt.rearrange("(k p) n -> p k n", p=P)
            for k in range(KD):
                nc.sync.dma_start(out=xt_t[:, k, :], in_=xt_r[:, k, :])

            # logits [tok, E] per 128-token group; k-major issue order so
            # the first xt k-tiles are consumed as they land
            psl = [ps.tile([P, E], F32, tag=f"psl{m}", name=f"psl{m}")
                   for m in range(M4)]
            for k in range(KD):
                for m in range(M4):
                    nc.tensor.matmul(psl[m][:], xt_t[:, k, m * P:(m + 1) * P],
                                     rwt_t[:, k, :],
                                     start=(k == 0), stop=(k == KD - 1))
            lg_all = wk.tile([P, M4, E], F32)
            for m in range(M4):
                nc.vector.tensor_copy(lg_all[:, m, :], psl[m][:])

            # batched top-2 softmax combine weights over all 512 tokens
            m1 = wk.tile([P, M4, 1], F32)
            nc.vector.tensor_reduce(m1[:], lg_all[:], axis=mybir.AxisListType.X,
                                    op=mybir.AluOpType.max)
            msk1 = wk.tile([P, M4, E], F32)
            nc.vector.tensor_tensor(out=msk1[:], in0=lg_all[:],
                                    in1=m1[:].to_broadcast([P, M4, E]),
                                    op=mybir.AluOpType.is_equal)
            lg2 = wk.tile([P, M4, E], F32)
            nc.vector.scalar_tensor_tensor(
                out=lg2[:], in0=msk1[:], scalar=-1e30, in1=lg_all[:],
                op0=mybir.AluOpType.mult, op1=mybir.AluOpType.add)
            m2 = wk.tile([P, M4, 1], F32)
            nc.vector.tensor_reduce(m2[:], lg2[:], axis=mybir.AxisListType.X,
                                    op=mybir.AluOpType.max)
            lgm = wk.tile([P, M4, E], F32)
            nc.vector.tensor_sub(lgm[:], lg_all[:], m1[:].to_broadcast([P, M4, E]))
            et = wk.tile([P, M4, E], F32)
            nc.scalar.activation(et[:], lgm[:], AF.Exp)
            edm = wk.tile([P, M4, 1], F32)
            nc.vector.tensor_sub(edm[:], m2[:], m1[:])
            ed = wk.tile([P, M4, 1], F32)
            nc.scalar.activation(ed[:], edm[:], AF.Exp)
            nc.vector.tensor_scalar_add(ed[:], ed[:], 1.0)
            rcp = wk.tile([P, M4, 1], F32)
            nc.vector.reciprocal(rcp[:], ed[:])
            msk = wk.tile([P, M4, E], F32)
            nc.vector.tensor_tensor(out=msk[:], in0=lg_all[:],
                                    in1=m2[:].to_broadcast([P, M4, E]),
                                    op=mybir.AluOpType.is_ge)
            wout = wk.tile([P, M4, E], F32)
            nc.vector.tensor_mul(wout[:], et[:], msk[:])
            nc.vector.tensor_mul(wout[:], wout[:], rcp[:].to_broadcast([P, M4, E]))
            nc.sync.dma_start(out=dw.rearrange("(m p) e -> p m e", p=P), in_=wout[:])
    nc.compile()
    return nc


def _build_expert(cap):
    """Per core (expert e): host-gathered, transposed, bf16 routed tokens
    xgT [D, cap] -> SwiGLU in bf16 -> scale by combine weight -> yt [D, cap] f32."""
    nc = bacc.Bacc(None, target_bir_lowering=False)
    xgT = nc.dram_tensor("xgT", [D, cap], BF16, kind="ExternalInput")
    wg = nc.dram_tensor("wg", [D, H], BF16, kind="ExternalInput")
    wu = nc.dram_tensor("wu", [D, H], BF16, kind="ExternalInput")
    wd = nc.dram_tensor("wd", [H, D], BF16, kind="ExternalInput")
    wtb = nc.dram_tensor("wtb", [P, cap], F32, kind="ExternalInput")
    yt = nc.dram_tensor("yt", [D, cap], F32, kind="ExternalOutput")

    chunks = _chunks_of(cap)
    (c0_n0, c0_nsz) = chunks[0]

    with tile.TileContext(nc) as tc:
        with tc.tile_pool(name="wts", bufs=1) as wts, \
             tc.tile_pool(name="ap", bufs=3) as ap, \
             tc.tile_pool(name="ps_gu", bufs=2, space="PSUM") as ps_gu, \
             tc.tile_pool(name="ps_y", bufs=1, space="PSUM") as ps_y:

            # PE warmup fillers: keep the PE busy from t=0 so the HAM clock
            # ramps while the first DMAs land.
            warm = wts.tile([P, 384], BF16)
            nc.gpsimd.memset(warm[:], 0.0)
            for w in range(5):
                psw = ps_gu.tile([P, 384], F32, tag="psg", name=f"psw{w}")
                nc.tensor.matmul(psw[:], warm[:, :P], warm[:], start=True, stop=True)

            wg_t = wts.tile([P, KD, H], BF16)
            wu_t = wts.tile([P, KD, H], BF16)
            wd_t = wts.tile([P, KH, D], BF16)
            xgT_t = wts.tile([P, KD, cap], BF16)
            wtb_t = wts.tile([P, cap], F32)
            wg_r = wg.rearrange("(k p) h -> p k h", p=P)
            wu_r = wu.rearrange("(k p) h -> p k h", p=P)
            wd_r = wd.rearrange("(k p) d -> p k d", p=P)
            xgT_r = xgT.rearrange("(k p) n -> p k n", p=P)

            # DMA issue order tracks first use: chunk-0 tokens, then the
            # h0 weight slices, then the rest in h order.
            for k in range(KD):
                nc.sync.dma_start(out=xgT_t[:, k, c0_n0:c0_n0 + c0_nsz],
                                  in_=xgT_r[:, k, c0_n0:c0_n0 + c0_nsz])
            for h in range(KH):
                hs = slice(h * P, (h + 1) * P)
                nc.sync.dma_start(out=wg_t[:, :, hs], in_=wg_r[:, :, hs])
                nc.sync.dma_start(out=wu_t[:, :, hs], in_=wu_r[:, :, hs])
                nc.sync.dma_start(out=wd_t[:, h, :], in_=wd_r[:, h, :])
            for (n0, nsz) in chunks[1:]:
                for k in range(KD):
                    nc.sync.dma_start(out=xgT_t[:, k, n0:n0 + nsz],
                                      in_=xgT_r[:, k, n0:n0 + nsz])
            nc.sync.dma_start(out=wtb_t[:], in_=wtb[:, :])

            for (n0, nsz) in chunks:
                psy = [ps_y.tile([P, nsz], F32, tag=f"psy{d}", name=f"psy{d}_{n0}")
                       for d in range(KD)]
                for h in range(KH):
                    psg = ps_gu.tile([P, nsz], F32, tag="psg")
                    psu = ps_gu.tile([P, nsz], F32, tag="psu")
                    for k in range(KD):
                        nc.tensor.matmul(
                            psg[:], wg_t[:, k, h * P:(h + 1) * P],
                            xgT_t[:, k, n0:n0 + nsz],
                            start=(k == 0), stop=(k == KD - 1))
                    for k in range(KD):
                        nc.tensor.matmul(
                            psu[:], wu_t[:, k, h * P:(h + 1) * P],
                            xgT_t[:, k, n0:n0 + nsz],
                            start=(k == 0), stop=(k == KD - 1))
                    actg = ap.tile([P, nsz], F32, tag="actg")
                    nc.scalar.activation(actg[:], psg[:], AF.Silu)
                    act = ap.tile([P, nsz], BF16, tag="act")
                    nc.vector.tensor_mul(act[:], actg[:], psu[:])
                    for d in range(KD):
                        nc.tensor.matmul(
                            psy[d][:], wd_t[:, h, d * P:(d + 1) * P], act[:],
                            start=(h == 0), stop=(h == KH - 1))
                for d in range(KD):
                    yts = ap.tile([P, nsz], F32, tag="yts")
                    nc.vector.tensor_mul(yts[:], psy[d][:], wtb_t[:, n0:n0 + nsz])
                    nc.sync.dma_start(out=yt[d * P:(d + 1) * P, n0:n0 + nsz],
                                      in_=yts[:])
    nc.compile()
    return nc


def _get_router_nc():
    global _router_nc
    if _router_nc is None:
        _router_nc = _build_router()
    return _router_nc


def _get_expert_nc(cap):
    if cap not in _expert_nc:
        _expert_nc[cap] = _build_expert(cap)
    return _expert_nc[cap]


def kernel(x, router_w, w_gate, w_up, w_down, _timings=None):
    x = np.ascontiguousarray(x, dtype=np.float32)
    router_w = np.ascontiguousarray(router_w, dtype=np.float32)

    flat = x.reshape(S_TOT, D)
    rwt = np.ascontiguousarray(router_w.T)  # [D, E]

    # ---- Launch A: router (data-parallel over token shards) ----
    nc_a = _get_router_nc()
    in_maps_a = []
    for c in range(N_CORES):
        sh = flat[c * SHARD:(c + 1) * SHARD]  # [SHARD, D]
        in_maps_a.append({"xt": np.ascontiguousarray(sh.T), "rwt": rwt})
    res_a = run_bass_kernel_spmd(nc_a, in_maps_a, core_ids=list(range(N_CORES)))
    dw = np.concatenate([res_a.results[c]["dw"] for c in range(N_CORES)], axis=0)
    if _timings is not None:
        _timings["router_ns"] = res_a.exec_time_ns

    # ---- Host: build the dispatch (the all-to-all by expert) ----
    sel = dw > 0.0
    idx_list = [np.nonzero(sel[:, e])[0].astype(np.int32) for e in range(E)]
    counts = [len(ix) for ix in idx_list]
    cap = max(max(counts), 1)
    cap = ((cap + P - 1) // P) * P

    flat_bf = flat.astype(NP_BF16)
    wg_bf = np.asarray(w_gate, dtype=NP_BF16)
    wu_bf = np.asarray(w_up, dtype=NP_BF16)
    wd_bf = np.asarray(w_down, dtype=NP_BF16)

    in_maps_b = []
    for e in range(E):
        ix = idx_list[e]
        xg = np.zeros((cap, D), dtype=NP_BF16)
        xg[:len(ix)] = flat_bf[ix]
        wt = np.zeros(cap, dtype=np.float32)
        wt[:len(ix)] = dw[ix, e]
        in_maps_b.append({
            "xgT": np.ascontiguousarray(xg.T),
            "wg": np.ascontiguousarray(wg_bf[e]),
            "wu": np.ascontiguousarray(wu_bf[e]),
            "wd": np.ascontiguousarray(wd_bf[e]),
            "wtb": np.ascontiguousarray(np.broadcast_to(wt[None, :], (P, cap))),
        })

    # ---- Launch B: experts (expert-parallel) ----
    nc_b = _get_expert_nc(cap)
    res_b = run_bass_kernel_spmd(nc_b, in_maps_b, core_ids=list(range(N_CORES)))
    if _timings is not None:
        _timings["expert_ns"] = res_b.exec_time_ns

    # ---- Host: combine (scatter-add back, then unshard) ----
    out = np.zeros((S_TOT, D), dtype=np.float32)
    for e in range(E):
        ix = idx_list[e]
        if len(ix) == 0:
            continue
        out[ix] += res_b.results[e]["yt"][:, :len(ix)].T  # indices unique per expert
    return out.reshape(B, S, D)
